# revision 14
# baseline (speedup 1.0000x reference)
"""Grouped-linear (EvolvedLoopLinear) Trainium2 Bass kernel.

Problem: out[b, j] = sum_s x[b, g*64+s] * weight[j, g*64+s] + bias[j],
with g = j % 128, for x [4096, 8192], weight [4096, 8192], bias [4096].

Strategy: data-parallel over batch across 8 cores (512 rows each).

The host pre-transposes each core's x shard to x^T and downcasts to
fp16, so the contraction dim (s) arrives on SBUF partitions directly
from DRAM — no PE transposes at all.  The host also gathers the live
weight slices (only 1 MiB of the 128 MiB weight contributes) into
block-diagonal per-group-pair stationaries, and lays x^T out
slab-major so every DMA moves 8 KiB contiguous per partition row.

Per core (batch shard of 512 = the matmul moving free dim N):
  - 64 group pairs; pair P covers groups (2P, 2P+1).  8 slabs of 8
    pairs; slab t's load is one [128, 4096] fp16 tile (1 MiB, 8 KiB
    per partition contiguous).
  - Quad q = pairs (2q, 2q+1): two matmuls with [128, 128] zero-padded
    block-diagonal stationaries accumulate into one [128, 512] PSUM
    bank; pair 2q's 64 outputs land on partitions 0-63, pair 2q+1's on
    64-127 (psum partition 64u + 32h + m <-> j = m*128 + 4q + 2u + h).
  - ACT evacuates psum with fused per-partition bias and fp32->fp16
    downcast into a [128, 2048] out tile (4 quads); one 1 MiB store
    per slab.  Host un-permutes and upcasts.
  - Weights load in 4 chunks on the store ring so the first matmul is
    gated only on chunk 0, not the full 2 MiB.
HBM traffic/core: 8 MiB x + 2 MiB w + 4 MiB out = 14 MiB (vs 25 fp32).
"""
import numpy as np
from contextlib import ExitStack

import concourse.bass as bass
import concourse.tile as tile
import concourse.tile_sem_assignment as _tsa
from concourse import bacc, mybir
from concourse.bass_utils import run_bass_kernel_spmd

# HWDGE completion lanes = max concurrent in-flight HWDGE DMAs (the Tile
# scheduler serializes DMAs within a lane to keep sem ticks monotone).  The
# old baseline had to cap this at 2 to keep its kernel-tail drain under the
# walrus per-instruction sem-wait limit; this kernel's small instruction
# count compiles fine with all 8, and 8 in-flight DMAs are what keep the
# load stream continuous.
import os as _os0
_tsa.NUM_HWDGE_SEMS = int(_os0.environ.get("K_HWSEMS", "8"))

# compact [128, 64] stationaries + PE column tiling (pair B's outputs land
# on PSUM partitions 64-127 via tile_position=(0, 64)) instead of
# zero-padded [128, 128] stationaries: halves the weight DMA to 1 MiB.
# DO NOT ENABLE: wedges the device (NRT_EXEC_UNIT_UNRECOVERABLE) on this
# walrus/runtime build — kept only as a record of the attempt.
W_COMPACT = _os0.environ.get("K_WCOMPACT", "0") == "1"

BATCH = 4096
IN_F = 8192
OUT_F = 4096
GROUPS = 128
STEP = 64
M_PER_G = 32          # outputs per group
N_CORES = 8
B_CORE = BATCH // N_CORES      # 512
N_PAIR = GROUPS // 2           # 64 group pairs
N_QUAD = GROUPS // 4           # 32 quads (2 pairs -> one psum bank)
N_SLAB = 8                     # 8 pairs per slab

f32 = mybir.dt.float32
f16 = mybir.dt.float16

_COMPILED = {}


def _build():
    if "nc" in _COMPILED:
        return _COMPILED["nc"]

    nc = bacc.Bacc("TRN2", target_bir_lowering=False, debug=False)
    WM = 64 if W_COMPACT else 128      # stationary column count per pair
    x_ap = nc.dram_tensor("xt_s", [N_SLAB * 128, 8 * B_CORE], f16,
                          kind="ExternalInput").ap()
    w_ap = nc.dram_tensor("w_bd", [128, N_PAIR * WM], f16,
                          kind="ExternalInput").ap()
    b_ap = nc.dram_tensor("bias_q", [128, N_QUAD], f32,
                          kind="ExternalInput").ap()
    y_ap = nc.dram_tensor("out_s", [N_SLAB * 128, 4 * B_CORE], f16,
                          kind="ExternalOutput").ap()

    with tile.TileContext(nc) as tc:
        with ExitStack() as ctx:
            const_pool = ctx.enter_context(tc.tile_pool(name="const", bufs=1))
            x_pool = ctx.enter_context(tc.tile_pool(name="xp", bufs=8))
            o_pool = ctx.enter_context(tc.tile_pool(name="op", bufs=6))
            ps_pool = ctx.enter_context(tc.tile_pool(name="ps", bufs=6, space="PSUM"))

            # weight chunk 0 goes FIRST on the sync ring so the first
            # matmuls are gated only on it + x slab 0; the remaining chunks
            # ride the ACT ring behind the (tiny) bias load.
            WCH = N_PAIR * WM // 4
            w_sb = const_pool.tile([128, N_PAIR * WM], f16)
            nc.sync.dma_start(w_sb[:, 0:WCH], w_ap[:, 0:WCH])
            bias_sb = const_pool.tile([128, N_QUAD], f32)
            nc.scalar.dma_start(bias_sb[:], b_ap[:])
            for wc in range(1, 4):
                nc.scalar.dma_start(w_sb[:, WCH * wc:WCH * (wc + 1)],
                                    w_ap[:, WCH * wc:WCH * (wc + 1)])

            for t in range(N_SLAB):
                xq = x_pool.tile([128, 8 * B_CORE], f16, tag="xp")
                nc.sync.dma_start(xq[:], x_ap[128 * t:128 * t + 128, :])
                ob = o_pool.tile([128, 4 * B_CORE], f16, tag="op")
                for uq in range(4):
                    q = 4 * t + uq         # quad index
                    ps = ps_pool.tile([128, B_CORE], f32, tag="ps")
                    for v in range(2):
                        k = 2 * uq + v     # pair within slab
                        P = 8 * t + k      # global pair index
                        mv = xq[:, k * B_CORE:(k + 1) * B_CORE]
                        if W_COMPACT:
                            # pair A -> PSUM partitions 0-63 (array col
                            # tile T0), pair B -> 64-127 (T1)
                            nc.tensor.matmul(
                                ps[64 * v:64 * v + 64, :],
                                w_sb[:, P * WM:(P + 1) * WM], mv,
                                start=True, stop=True,
                                tile_position=(0, 64 * v))
                        else:
                            nc.tensor.matmul(
                                ps[:], w_sb[:, P * WM:(P + 1) * WM], mv,
                                start=(v == 0), stop=(v == 1))
                    # alternate ACT/DVE for the psum evac: halves the
                    # serial evac chain (they hit different PSUM banks)
                    dst = ob[:, uq * B_CORE:(uq + 1) * B_CORE]
                    if uq % 2 == 0:
                        nc.scalar.add(dst, ps[:], bias_sb[:, q:q + 1])
                    else:
                        nc.vector.tensor_scalar_add(dst, ps[:],
                                                    bias_sb[:, q:q + 1])
                # stores ride the SWDGE (gpsimd) path: HWDGE completion
                # lanes are round-robin over all HWDGE DMAs with a 1-per-lane
                # in-flight cap, so an evac-blocked store on a shared lane
                # stalls a later x load.  SWDGE has its own 8 lanes.
                nc.gpsimd.dma_start(y_ap[128 * t:128 * t + 128, :], ob[:])

    nc.compile()
    _COMPILED["nc"] = nc
    return nc


def _host_prep(weight, bias):
    # gather: Wg[j, s] = weight[j, (j%128)*64 + s]
    j = np.arange(OUT_F)
    Wg = weight.reshape(OUT_F, GROUPS, STEP)[j, j % GROUPS]      # [4096, 64]
    W4 = Wg.reshape(M_PER_G, GROUPS, STEP)                       # [m, g, s]
    Wk = W4.reshape(M_PER_G, N_PAIR, 2, STEP)                    # [m, p, h, s]
    if W_COMPACT:
        # block-diagonal [128, 64] stationary per pair:
        # w_bd[64h + s, 64p + 32h' + m] = Wk[m, p, h, s] iff h==h'
        # (the psum half u = p%2 comes from tile_position, not padding)
        w_bd = np.zeros((2, STEP, N_PAIR, 64), dtype=np.float16)
        for h in range(2):
            blk = Wk[:, :, h, :].transpose(2, 1, 0).astype(np.float16)  # [s, p, m]
            w_bd[h, :, :, 32 * h:32 * h + M_PER_G] = blk
        w_bd = np.ascontiguousarray(w_bd.reshape(128, N_PAIR * 64))
    else:
        # stationary for pair p, zero-padded to M=128 for the quad scheme:
        # w_bd[64h + s, 128p + 64u + 32h' + m] = Wk[m, p, h, s] iff h==h',
        # u = p % 2 (which half of the quad's psum partitions it lands on)
        w_bd = np.zeros((2, STEP, N_PAIR, 128), dtype=np.float16)   # [h, s, p, M]
        u = (np.arange(N_PAIR) % 2)                                 # [p]
        for h in range(2):
            blk = Wk[:, :, h, :].transpose(2, 1, 0).astype(np.float16)  # [s, p, m]
            for p in range(N_PAIR):
                w_bd[h, :, p, 64 * u[p] + 32 * h: 64 * u[p] + 32 * h + M_PER_G] = blk[:, p, :]
        w_bd = np.ascontiguousarray(w_bd.reshape(128, N_PAIR * 128))

    # bias in quad psum layout: bias_q[64u + 32h + m, q] = bias[m*128 + 4q + 2u + h]
    bq = bias.reshape(M_PER_G, N_QUAD, 2, 2)                     # [m, q, u, h]
    bias_q = bq.transpose(2, 3, 0, 1).reshape(128, N_QUAD)       # [(u h m), q]
    bias_q = np.ascontiguousarray(bias_q.astype(np.float32))
    return w_bd, bias_q


def _make_in_maps(inputs):
    x = np.asarray(inputs["x"], dtype=np.float32)
    weight = np.asarray(inputs["weight"], dtype=np.float32)
    bias = np.asarray(inputs["bias"], dtype=np.float32)
    w_bd, bias_q = _host_prep(weight, bias)
    in_maps = []
    for c in range(N_CORES):
        xt = x[c * B_CORE:(c + 1) * B_CORE].T.astype(np.float16)  # [8192, 512]
        # slab-major: x_dram[128t + p, 512k + c] = xt[1024t + 128k + p, c]
        xs = np.ascontiguousarray(
            xt.reshape(N_SLAB, 8, 128, B_CORE).transpose(0, 2, 1, 3)
            .reshape(N_SLAB * 128, 8 * B_CORE))
        in_maps.append({"xt_s": xs, "w_bd": w_bd, "bias_q": bias_q})
    return in_maps


def _unpermute(y):
    # y [1024, 2048] fp16: y[128t + (64u + 32h + m), 512uq + c]
    #   -> j = m*128 + 16t + 4uq + 2u + h, b = c
    y6 = y.reshape(N_SLAB, 2, 2, M_PER_G, 4, B_CORE)     # [t, u, h, m, uq, c]
    o = y6.transpose(3, 0, 4, 1, 2, 5).reshape(OUT_F, B_CORE)  # [(m t uq u h), c]
    return np.ascontiguousarray(o.T.astype(np.float32))        # [512, 4096]


def kernel(x, weight, bias):
    nc = _build()
    in_maps = _make_in_maps({"x": x, "weight": weight, "bias": bias})
    res = run_bass_kernel_spmd(nc, in_maps, core_ids=list(range(N_CORES)))
    out = np.concatenate(
        [_unpermute(res.results[c]["out_s"]) for c in range(N_CORES)], axis=0)
    return out


# revision 15
# speedup vs baseline: 1.0978x; 1.0978x over previous
"""Grouped-linear (EvolvedLoopLinear) Trainium2 Bass kernel.

Problem: out[b, j] = sum_s x[b, g*64+s] * weight[j, g*64+s] + bias[j],
with g = j % 128, for x [4096, 8192], weight [4096, 8192], bias [4096].

Strategy: data-parallel over batch across 8 cores (512 rows each).

The host pre-transposes each core's x shard to x^T and downcasts to
fp16, so the contraction dim (s) arrives on SBUF partitions directly
from DRAM — no PE transposes at all.  The host also gathers the live
weight slices (only 1 MiB of the 128 MiB weight contributes) into
block-diagonal per-group-pair stationaries, and lays x^T out
slab-major so every DMA moves 8 KiB contiguous per partition row.

Per core (batch shard of 512 = the matmul moving free dim N):
  - 64 group pairs; pair P covers groups (2P, 2P+1).  8 slabs of 8
    pairs; slab t's load is one [128, 4096] fp16 tile (1 MiB, 8 KiB
    per partition contiguous).
  - Quad q = pairs (2q, 2q+1): two matmuls with [128, 128] zero-padded
    block-diagonal stationaries accumulate into one [128, 512] PSUM
    bank; pair 2q's 64 outputs land on partitions 0-63, pair 2q+1's on
    64-127 (psum partition 64u + 32h + m <-> j = m*128 + 4q + 2u + h).
  - ACT evacuates psum with fused per-partition bias and fp32->fp16
    downcast into a [128, 2048] out tile (4 quads); one 1 MiB store
    per slab.  Host un-permutes and upcasts.
  - Weights load in 4 chunks on the store ring so the first matmul is
    gated only on chunk 0, not the full 2 MiB.
HBM traffic/core: 8 MiB x + 2 MiB w + 4 MiB out = 14 MiB (vs 25 fp32).
"""
import numpy as np
from contextlib import ExitStack

import concourse.bass as bass
import concourse.tile as tile
import concourse.tile_sem_assignment as _tsa
from concourse import bacc, mybir
from concourse.bass_utils import run_bass_kernel_spmd

# HWDGE completion lanes = max concurrent in-flight HWDGE DMAs (the Tile
# scheduler serializes DMAs within a lane to keep sem ticks monotone).  The
# old baseline had to cap this at 2 to keep its kernel-tail drain under the
# walrus per-instruction sem-wait limit; this kernel's small instruction
# count compiles fine with all 8, and 8 in-flight DMAs are what keep the
# load stream continuous.
import os as _os0
_tsa.NUM_HWDGE_SEMS = int(_os0.environ.get("K_HWSEMS", "8"))

# compact [128, 64] stationaries + PE column tiling (pair B's outputs land
# on PSUM partitions 64-127 via tile_position=(0, 64)) instead of
# zero-padded [128, 128] stationaries: halves the weight DMA to 1 MiB.
# DO NOT ENABLE: wedges the device (NRT_EXEC_UNIT_UNRECOVERABLE) on this
# walrus/runtime build — kept only as a record of the attempt.
W_COMPACT = _os0.environ.get("K_WCOMPACT", "0") == "1"

BATCH = 4096
IN_F = 8192
OUT_F = 4096
GROUPS = 128
STEP = 64
M_PER_G = 32          # outputs per group
N_CORES = 8
B_CORE = BATCH // N_CORES      # 512
N_PAIR = GROUPS // 2           # 64 group pairs
N_QUAD = GROUPS // 4           # 32 quads (2 pairs -> one psum bank)
PPS = int(_os0.environ.get("K_PPS", "4"))   # pairs per slab
N_SLAB = N_PAIR // PPS
QPS = PPS // 2                 # quads per slab

f32 = mybir.dt.float32
f16 = mybir.dt.float16

_COMPILED = {}


def _build():
    if "nc" in _COMPILED:
        return _COMPILED["nc"]

    nc = bacc.Bacc("TRN2", target_bir_lowering=False, debug=False)
    WM = 64 if W_COMPACT else 128      # stationary column count per pair
    x_ap = nc.dram_tensor("xt_s", [N_SLAB * 128, PPS * B_CORE], f16,
                          kind="ExternalInput").ap()
    w_ap = nc.dram_tensor("w_bd", [128, N_PAIR * WM], f16,
                          kind="ExternalInput").ap()
    b_ap = nc.dram_tensor("bias_q", [128, N_QUAD], f32,
                          kind="ExternalInput").ap()
    y_ap = nc.dram_tensor("out_s", [N_SLAB * 128, QPS * B_CORE], f16,
                          kind="ExternalOutput").ap()

    with tile.TileContext(nc) as tc:
        with ExitStack() as ctx:
            const_pool = ctx.enter_context(tc.tile_pool(name="const", bufs=1))
            x_pool = ctx.enter_context(tc.tile_pool(name="xp", bufs=N_SLAB))
            o_pool = ctx.enter_context(tc.tile_pool(name="op", bufs=6))
            ps_pool = ctx.enter_context(tc.tile_pool(name="ps", bufs=6, space="PSUM"))

            # weight chunk 0 goes FIRST on the sync ring so the first
            # matmuls are gated only on it + x slab 0; the remaining chunks
            # ride the ACT ring behind the (tiny) bias load.
            WCH = N_PAIR * WM // 4
            w_sb = const_pool.tile([128, N_PAIR * WM], f16)
            nc.sync.dma_start(w_sb[:, 0:WCH], w_ap[:, 0:WCH])
            bias_sb = const_pool.tile([128, N_QUAD], f32)
            nc.scalar.dma_start(bias_sb[:], b_ap[:])
            for wc in range(1, 4):
                nc.scalar.dma_start(w_sb[:, WCH * wc:WCH * (wc + 1)],
                                    w_ap[:, WCH * wc:WCH * (wc + 1)])

            for t in range(N_SLAB):
                xq = x_pool.tile([128, PPS * B_CORE], f16, tag="xp")
                nc.sync.dma_start(xq[:], x_ap[128 * t:128 * t + 128, :])
                ob = o_pool.tile([128, QPS * B_CORE], f16, tag="op")
                for uq in range(QPS):
                    q = QPS * t + uq       # quad index
                    ps = ps_pool.tile([128, B_CORE], f32, tag="ps")
                    for v in range(2):
                        k = 2 * uq + v     # pair within slab
                        P = PPS * t + k    # global pair index
                        mv = xq[:, k * B_CORE:(k + 1) * B_CORE]
                        if W_COMPACT:
                            # pair A -> PSUM partitions 0-63 (array col
                            # tile T0), pair B -> 64-127 (T1)
                            nc.tensor.matmul(
                                ps[64 * v:64 * v + 64, :],
                                w_sb[:, P * WM:(P + 1) * WM], mv,
                                start=True, stop=True,
                                tile_position=(0, 64 * v))
                        else:
                            nc.tensor.matmul(
                                ps[:], w_sb[:, P * WM:(P + 1) * WM], mv,
                                start=(v == 0), stop=(v == 1))
                    # alternate ACT/DVE for the psum evac: halves the
                    # serial evac chain (they hit different PSUM banks)
                    dst = ob[:, uq * B_CORE:(uq + 1) * B_CORE]
                    if uq % 2 == 0:
                        nc.scalar.add(dst, ps[:], bias_sb[:, q:q + 1])
                    else:
                        nc.vector.tensor_scalar_add(dst, ps[:],
                                                    bias_sb[:, q:q + 1])
                # one store per slab on the ACT HWDGE ring: program order
                # alternates load,store 1:1, so HWDGE completion lanes
                # (1 in-flight DMA each) split evenly between the streams.
                # (Tested worse: per-2-quad stores [lane conflicts, 56.1us],
                # SWDGE gpsimd stores [56.2us] vs 49.7us for this shape.)
                nc.scalar.dma_start(y_ap[128 * t:128 * t + 128, :], ob[:])

    nc.compile()
    _COMPILED["nc"] = nc
    return nc


def _host_prep(weight, bias):
    # gather: Wg[j, s] = weight[j, (j%128)*64 + s]
    j = np.arange(OUT_F)
    Wg = weight.reshape(OUT_F, GROUPS, STEP)[j, j % GROUPS]      # [4096, 64]
    W4 = Wg.reshape(M_PER_G, GROUPS, STEP)                       # [m, g, s]
    Wk = W4.reshape(M_PER_G, N_PAIR, 2, STEP)                    # [m, p, h, s]
    if W_COMPACT:
        # block-diagonal [128, 64] stationary per pair:
        # w_bd[64h + s, 64p + 32h' + m] = Wk[m, p, h, s] iff h==h'
        # (the psum half u = p%2 comes from tile_position, not padding)
        w_bd = np.zeros((2, STEP, N_PAIR, 64), dtype=np.float16)
        for h in range(2):
            blk = Wk[:, :, h, :].transpose(2, 1, 0).astype(np.float16)  # [s, p, m]
            w_bd[h, :, :, 32 * h:32 * h + M_PER_G] = blk
        w_bd = np.ascontiguousarray(w_bd.reshape(128, N_PAIR * 64))
    else:
        # stationary for pair p, zero-padded to M=128 for the quad scheme:
        # w_bd[64h + s, 128p + 64u + 32h' + m] = Wk[m, p, h, s] iff h==h',
        # u = p % 2 (which half of the quad's psum partitions it lands on)
        w_bd = np.zeros((2, STEP, N_PAIR, 128), dtype=np.float16)   # [h, s, p, M]
        u = (np.arange(N_PAIR) % 2)                                 # [p]
        for h in range(2):
            blk = Wk[:, :, h, :].transpose(2, 1, 0).astype(np.float16)  # [s, p, m]
            for p in range(N_PAIR):
                w_bd[h, :, p, 64 * u[p] + 32 * h: 64 * u[p] + 32 * h + M_PER_G] = blk[:, p, :]
        w_bd = np.ascontiguousarray(w_bd.reshape(128, N_PAIR * 128))

    # bias in quad psum layout: bias_q[64u + 32h + m, q] = bias[m*128 + 4q + 2u + h]
    bq = bias.reshape(M_PER_G, N_QUAD, 2, 2)                     # [m, q, u, h]
    bias_q = bq.transpose(2, 3, 0, 1).reshape(128, N_QUAD)       # [(u h m), q]
    bias_q = np.ascontiguousarray(bias_q.astype(np.float32))
    return w_bd, bias_q


def _make_in_maps(inputs):
    x = np.asarray(inputs["x"], dtype=np.float32)
    weight = np.asarray(inputs["weight"], dtype=np.float32)
    bias = np.asarray(inputs["bias"], dtype=np.float32)
    w_bd, bias_q = _host_prep(weight, bias)
    in_maps = []
    for c in range(N_CORES):
        xt = x[c * B_CORE:(c + 1) * B_CORE].T.astype(np.float16)  # [8192, 512]
        # slab-major: x_dram[128t + p, 512k + c] = xt[128(PPS*t + k) + p, c]
        xs = np.ascontiguousarray(
            xt.reshape(N_SLAB, PPS, 128, B_CORE).transpose(0, 2, 1, 3)
            .reshape(N_SLAB * 128, PPS * B_CORE))
        in_maps.append({"xt_s": xs, "w_bd": w_bd, "bias_q": bias_q})
    return in_maps


def _unpermute(y):
    # y [N_SLAB*128, QPS*512] fp16: y[128t + (64u + 32h + m), 512uq + c]
    #   -> j = m*128 + 4*(QPS*t + uq) + 2u + h, b = c
    y6 = y.reshape(N_SLAB, 2, 2, M_PER_G, QPS, B_CORE)   # [t, u, h, m, uq, c]
    o = y6.transpose(3, 0, 4, 1, 2, 5).reshape(OUT_F, B_CORE)  # [(m t uq u h), c]
    return np.ascontiguousarray(o.T.astype(np.float32))        # [512, 4096]


def kernel(x, weight, bias):
    nc = _build()
    in_maps = _make_in_maps({"x": x, "weight": weight, "bias": bias})
    res = run_bass_kernel_spmd(nc, in_maps, core_ids=list(range(N_CORES)))
    out = np.concatenate(
        [_unpermute(res.results[c]["out_s"]) for c in range(N_CORES)], axis=0)
    return out
